# revision 35
# baseline (speedup 1.0000x reference)
"""Trainium2 Bass kernel for nn_GBF2DEncoder (gaussian-basis edge encoder).

Strategy
--------
The reference zeroes `attn` and `edge_features` wherever BOTH endpoint
atoms are valid (pair_mask True), so the only pairs (i, j) that ever
contribute to either output are those touching a padding atom (atom id
0).  With the graded inputs there are at most ~5 padding atoms per
batch, so per batch element only ~2*P*N of the N*N pairs matter.

Sharding: pure data parallel, one batch element per NeuronCore (B=8,
M=8 cores).  The host gathers the per-pair scalars (xv = mul*dist+bias)
for the needed pairs, and each core computes, for a fixed-size slab of
2*P*N pairs:

    t    = a_k * xv                (PE, fp32r hi/lo split -> exact)
    sq   = (t + b_k)^2             (ACT Square, per-partition bias)
    g    = exp(-0.5 * sq)          (ACT Exp)
    y    = W1' @ g                 (PE fp32r, W1' = l1_w * C_k folded)
    h    = gelu(y)                 (ACT, exact erf gelu)
    attn = W2 @ h                  (PE fp32r)
    row/col slab sums of g         (DVE reduce)
    dist_emb = Wep' @ sums         (PE fp32r, ep_w * C_k / 100 folded)

fp32r runs the PE at 1 cycle/row (vs 4 for fp32) and rounds operands to
~11 mantissa bits.  For the z computation that rounding is not
acceptable (cancellation magnitudes ~500 need abs error <1e-3), so a
and xv are each split into a 10-bit hi part and an fp32 lo remainder;
all four cross products are exactly representable, making a*xv exact to
fp32 accumulation.  b is added exactly by the ACT affine (fp32).

The host scatters the compact slab results into the full-shape zero
outputs.  If the inputs violate the preconditions baked into the
compiled kernel (more padding atoms than slab slots, non-zero MLP
biases, degenerate stds), a pure-numpy fallback computes the exact
reference semantics instead.
"""

import numpy as np

_B, _N, _K, _H, _D = 8, 256, 128, 32, 768
_P = 6                      # padding-atom slots per batch
_NPR = _P * _N              # row-slab pairs (i in padding, all j)
_NPAIR = 2 * _P * _N        # + col-slab pairs (all i, j in padding)
_CHUNK = 512
_BLK = 1024                 # ACT/psum block (2 matmul chunks)
_SQRT_2PI = (2.0 * 3.14159) ** 0.5   # module hardcodes pi=3.14159
_DUMMY_XV = np.float32(1.0e4)        # forces exp(-0.5 z^2) == 0 exactly

_CACHE = {}


def _round_mantissa(x, bits=10):
    """Round fp32 to `bits` mantissa bits (RNE-ish), so fp32r's ~11-bit
    operand rounding is an exact no-op on the result."""
    u = np.asarray(x, np.float32).view(np.uint32).astype(np.uint64)
    keep = 23 - bits
    u = (u + (1 << (keep - 1))) & ~np.uint64((1 << keep) - 1)
    return u.astype(np.uint32).view(np.float32)


# ----------------------------------------------------------------- device ---

def _build_bass():
    import concourse.bacc as bacc
    import concourse.mybir as mybir
    import concourse.tile as tile

    f32 = mybir.dt.float32
    f32r = mybir.dt.float32r
    AF = mybir.ActivationFunctionType
    AX = mybir.AxisListType
    ALU = mybir.AluOpType

    nc = bacc.Bacc("TRN2", target_bir_lowering=False, debug=False,
                   num_devices=_B)

    # wrhs: wg4 (cols 0:128) + rhs4 (cols 128:128+NPAIR), on partitions 0-3
    wrhs_d = nc.dram_tensor("wrhs", [4, _K + _NPAIR], f32r,
                            kind="ExternalInput")
    # wpack: w1t | w2t | wept along columns, 128 partitions
    WPX = _K + _H + _D
    wpack_d = nc.dram_tensor("wpack", [_K, WPX], f32r, kind="ExternalInput")
    bexp_d = nc.dram_tensor("bexp", [_K, 1], f32, kind="ExternalInput")
    attn_d = nc.dram_tensor("attn_d", [128, _NPAIR // 3], f32,
                            kind="ExternalOutput")
    de_d = nc.dram_tensor("de_d", [_N, _D], f32, kind="ExternalOutput")
    dep_d = nc.dram_tensor("dep_d", [_P, _D], f32, kind="ExternalOutput")

    HD = _D // 2
    NBLK = _NPAIR // _BLK

    with tile.TileContext(nc) as tc:
        with tc.tile_pool(name="const", bufs=1) as cpool, \
             tc.tile_pool(name="work", bufs=2) as wpool, \
             tc.tile_pool(name="big", bufs=1) as bpool, \
             tc.tile_pool(name="ps", bufs=2, space="PSUM") as ppool:

            wrhs_s = cpool.tile([4, _K + _NPAIR], f32r)
            nc.sync.dma_start(wrhs_s[:, 0:640], wrhs_d.ap()[:, 0:640])
            bexp_s = cpool.tile([_K, 1], f32)
            nc.sync.dma_start(bexp_s[:], bexp_d.ap())
            nc.sync.dma_start(wrhs_s[:, 640:1664], wrhs_d.ap()[:, 640:1664])
            nc.sync.dma_start(wrhs_s[:, 1664:3200], wrhs_d.ap()[:, 1664:3200])
            wpack_s = cpool.tile([_K, WPX], f32r)
            # w1t+w2t first (needed by phase B), wept later (phase C);
            # all behind the wrhs chunks so the z-path data wins the DMA race
            nc.sync.dma_start(wpack_s[:, 0:_K + _H], wpack_d.ap()[:, 0:_K + _H])
            nc.sync.dma_start(wpack_s[:, _K + _H:WPX],
                              wpack_d.ap()[:, _K + _H:WPX])
            wg4_s = wrhs_s[:, 0:_K]
            rhs4_s = wrhs_s[:, _K:_K + _NPAIR]
            w1t_s = wpack_s[:, 0:_K]
            w2t_s = wpack_s[:, _K:_K + _H]
            wept_s = wpack_s[:, _K + _H:_K + _H + _D]

            # dep-free dummy op: hosts the exp-set ACT_TABLE_LOAD so it
            # runs during the input DMAs instead of on the critical path
            dummy = cpool.tile([1, 1], f32)
            nc.scalar.activation(dummy[:], dummy[:], AF.Exp)
            # dep-free dummy matmuls: warm the PE (HAM clock gate releases
            # after ~3us of activity) while the input DMAs are in flight
            warm = cpool.tile([_K, 320], f32)
            nc.gpsimd.memset(warm[:], 0.0)
            w_ps = ppool.tile([_K, 320], f32, tag="ap", name="warm0")
            nc.tensor.matmul(w_ps[:], warm[:, 0:_K], warm[:],
                             start=True, stop=True)

            sq_all = bpool.tile([_K, _NPAIR], f32)
            g_all = bpool.tile([_K, _NPAIR], f32r)
            attn_sb = bpool.tile([128, _NPAIR // 3], f32)

            # phase A: t = a*xv (exact via hi/lo), sq = (t + b)^2, g = exp(-sq/2)
            # geometric block sizes: the ACT ladder starts as soon as the
            # first 512 pairs of z exist, at the same total ACT cost
            last_exp = None
            for bi, (p0, p1) in enumerate([(0, 512), (512, 1536),
                                           (1536, 3072)]):
                z_ps = ppool.tile([_K, 1536], f32, tag="zp", bufs=1,
                                  name=f"zp{bi}", padded_shape=None)
                for c0 in range(p0, p1, _CHUNK):
                    nc.tensor.matmul(z_ps[:, c0 - p0:c0 - p0 + _CHUNK],
                                     wg4_s[:], rhs4_s[:, c0:c0 + _CHUNK],
                                     start=True, stop=True)
                nc.scalar.activation(sq_all[:, p0:p1],
                                     z_ps[:, 0:p1 - p0], AF.Square,
                                     bias=bexp_s[:], scale=1.0)
                last_exp = nc.scalar.activation(g_all[:, p0:p1],
                                                sq_all[:, p0:p1],
                                                AF.Exp, scale=-0.5)

            # phase C: slab sums of g, then dist_emb projections
            rowsum = bpool.tile([_K, _P], f32)
            colsum = bpool.tile([_K, _N], f32)
            rview = g_all[:, 0:_NPR].bitcast(f32).rearrange(
                "p (s j) -> p s j", j=_N)
            cview = g_all[:, _NPR:_NPAIR].bitcast(f32).rearrange(
                "p (i t) -> p i t", t=_P)
            nc.vector.tensor_reduce(rowsum[:], rview, axis=AX.X, op=ALU.add)
            nc.vector.tensor_reduce(colsum[:], cview, axis=AX.X, op=ALU.add)
            rowsum_r = bpool.tile([_K, _P], f32r)
            colsum_r = bpool.tile([_K, _N], f32r)
            nc.vector.tensor_copy(rowsum_r[:], rowsum[:])
            nc.vector.tensor_copy(colsum_r[:], colsum[:])

            dep_sb = bpool.tile([_P, _D], f32)
            for dh in range(2):
                dp_ps = ppool.tile([_P, HD], f32, tag="ap", name=f"dpp{dh}")
                nc.tensor.matmul(dp_ps[:], rowsum_r[:],
                                 wept_s[:, dh * HD:(dh + 1) * HD],
                                 start=True, stop=True)
                nc.vector.tensor_copy(dep_sb[:, dh * HD:(dh + 1) * HD],
                                      dp_ps[:])
            nc.sync.dma_start(dep_d.ap()[:], dep_sb[:])

            act_copies = []
            for ih in range(2):
                de_sb = bpool.tile([128, _D], f32, name=f"de_sb{ih}")
                for dh in range(2):
                    d_ps = ppool.tile([128, HD], f32, tag="zp", bufs=1,
                                      name=f"dps{ih}{dh}")
                    nc.tensor.matmul(d_ps[:],
                                     colsum_r[:, ih * 128:(ih + 1) * 128],
                                     wept_s[:, dh * HD:(dh + 1) * HD],
                                     start=True, stop=True)
                    if ih == 0:
                        # DVE has a free window before the attn copies
                        nc.vector.tensor_copy(
                            de_sb[:, dh * HD:(dh + 1) * HD], d_ps[:])
                    else:
                        # ACT picks these up after its gelus drain
                        ci = nc.scalar.copy(
                            de_sb[:, dh * HD:(dh + 1) * HD], d_ps[:])
                        act_copies.append(ci)
                nc.sync.dma_start(de_d.ap()[ih * 128:(ih + 1) * 128, :],
                                  de_sb[:])

            # phase B: y = W1'.T @ g, h = gelu(y), attn = W2.T @ h
            gelu_insts = []
            NC = _NPAIR // _CHUNK
            for c in range(NC):
                sl = slice(c * _CHUNK, (c + 1) * _CHUNK)
                y_ps = ppool.tile([_K, _CHUNK], f32, tag="yp", bufs=3,
                                  name=f"yp{c}")
                nc.tensor.matmul(y_ps[:], w1t_s[:], g_all[:, sl],
                                 start=True, stop=True)
                h_sb = wpool.tile([_K, _CHUNK], f32r, tag="h", name=f"h{c}")
                gelu_inst = nc.scalar.activation(h_sb[:], y_ps[:], AF.Gelu)
                gelu_insts.append(gelu_inst)
                # keep every Gelu after the last Exp on ACT: table sets
                # differ, and an interleave costs a ~1.3us table reload each
                tile.add_dep_helper(gelu_inst.ins, last_exp.ins, sync=False,
                                    reason="group ACT ops by table set")
                a_ps = ppool.tile([_H, _CHUNK], f32, tag="ap", name=f"ap{c}")
                nc.tensor.matmul(a_ps[:], w2t_s[:], h_sb[:],
                                 start=True, stop=True)
                # pack 4 chunks per 512-col block across the 128 partitions
                # (partition-shifted copy) so the output DMA runs at full
                # port width
                nc.vector.tensor_copy(
                    attn_sb[_H * (c % 4):_H * (c % 4 + 1),
                            (c // 4) * _CHUNK:(c // 4 + 1) * _CHUNK],
                    a_ps[:])
            nc.sync.dma_start(attn_d.ap()[:, 0:_CHUNK],
                              attn_sb[:, 0:_CHUNK])
            nc.sync.dma_start(attn_d.ap()[:, _CHUNK:2 * _CHUNK],
                              attn_sb[:, _CHUNK:2 * _CHUNK])
            # keep the ACT-side de copies out of the gelu ladder
            for ci in act_copies:
                tile.add_dep_helper(ci.ins, gelu_insts[-1].ins, sync=False,
                                    reason="de copies after gelus on ACT")

    nc.compile()
    return nc


def _get_nc():
    if "nc" not in _CACHE:
        _CACHE["nc"] = _build_bass()
    return _CACHE["nc"]


# ------------------------------------------------------------------- host ---

def _dist_xv(pos_b, mw, bw, nte_b):
    """Full [N, N] xv = mul*dist + bias for one batch, float32 to mirror
    the reference's fp32 arithmetic."""
    delta = pos_b[None, :, :] - pos_b[:, None, :]          # [N,N,3] f32
    sq = (delta * delta).sum(axis=-1, dtype=np.float32)    # [N,N]
    small = sq < np.float32(1e-4)
    dist = np.where(small, np.float32(100.0),
                    np.sqrt(np.where(small, np.float32(1.0), sq)))
    mul = (mw[nte_b[:, :, 0]] + mw[nte_b[:, :, 1]]) * np.float32(0.5)
    bias = (bw[nte_b[:, :, 0]] + bw[nte_b[:, :, 1]]) * np.float32(0.5)
    return mul * dist + bias                               # [N,N] f32


def _erf(x):
    try:
        from scipy.special import erf
        return erf(x).astype(np.float32)
    except Exception:
        import math
        f = np.frompyfunc(math.erf, 1, 1)
        return f(x.astype(np.float64)).astype(np.float32)


def _fallback(pos, means, stds, mul_w, bias_w, l1_w, l1_b, l2_w, l2_b,
              ep_w, ep_b, x, node_type_edge):
    """Exact numpy replication of the reference for arbitrary inputs."""
    atoms = x[:, :, 0]
    valid = atoms != 0
    pair_mask = valid[:, None, :] & valid[:, :, None]      # [B,N,N]
    mw = mul_w[:, 0]
    bw = bias_w[:, 0]
    std = np.abs(stds) + np.float32(1e-5)
    B, N = atoms.shape
    K = means.shape[0]
    D = ep_w.shape[0]
    Hh = l2_w.shape[0]
    dist_emb = np.empty((B, N, D), np.float32)
    attn = np.empty((B, Hh, N, N), np.float32)
    for b in range(B):
        xv = _dist_xv(pos[b], mw, bw, node_type_edge[b])[:, :, None]
        z = (xv - means) / std                             # [N,N,K]
        gbf = np.exp(np.float32(-0.5) * z * z) / (np.float32(_SQRT_2PI) * std)
        ef = np.where(pair_mask[b][:, :, None], np.float32(0.0), gbf)
        y = gbf.reshape(-1, K) @ l1_w.T + l1_b
        h = (y * np.float32(0.5) * (1.0 + _erf(y / np.float32(np.sqrt(2.0)))))
        a = (h @ l2_w.T + l2_b).reshape(N, N, Hh)
        a = np.where(pair_mask[b][:, :, None], np.float32(0.0), a)
        attn[b] = np.transpose(a, (2, 0, 1))
        de = ef.sum(axis=1) @ ep_w.T + ep_b
        dist_emb[b] = de / np.float32(100.0)
    return dist_emb, attn


def kernel(pos, means, stds, mul_w, bias_w, l1_w, l1_b, l2_w, l2_b,
           ep_w, ep_b, x, node_type_edge):
    pos = np.asarray(pos, np.float32)
    means = np.asarray(means, np.float32)
    stds = np.asarray(stds, np.float32)
    mul_w = np.asarray(mul_w, np.float32)
    bias_w = np.asarray(bias_w, np.float32)
    l1_w = np.asarray(l1_w, np.float32)
    l1_b = np.asarray(l1_b, np.float32)
    l2_w = np.asarray(l2_w, np.float32)
    l2_b = np.asarray(l2_b, np.float32)
    ep_w = np.asarray(ep_w, np.float32)
    ep_b = np.asarray(ep_b, np.float32)
    x = np.asarray(x)
    node_type_edge = np.asarray(node_type_edge)

    atoms = x[:, :, 0]
    pad_lists = [np.nonzero(atoms[b] == 0)[0] for b in range(atoms.shape[0])]
    std = np.abs(stds) + np.float32(1e-5)

    device_ok = (
        atoms.shape == (_B, _N)
        and pos.shape == (_B, _N, 3)
        and node_type_edge.shape == (_B, _N, _N, 2)
        and means.shape == (_K,)
        and l1_w.shape == (_K, _K)
        and l2_w.shape == (_H, _K)
        and ep_w.shape == (_D, _K)
        and max(len(p) for p in pad_lists) <= _P
        and float(std.min()) >= 1e-3
        and not l1_b.any() and not l2_b.any() and not ep_b.any()
    )
    if not device_ok:
        return _fallback(pos, means, stds, mul_w, bias_w, l1_w, l1_b,
                         l2_w, l2_b, ep_w, ep_b, x, node_type_edge)

    mw = mul_w[:, 0]
    bw = bias_w[:, 0]
    a64 = 1.0 / std.astype(np.float64)
    C64 = 1.0 / (_SQRT_2PI * std.astype(np.float64))
    a32 = a64.astype(np.float32)
    a_hi = _round_mantissa(a32)
    a_lo = (a64 - a_hi.astype(np.float64)).astype(np.float32)

    wg4 = np.empty((4, _K), np.float32)
    wg4[0] = a_hi
    wg4[1] = a_hi
    wg4[2] = _round_mantissa(a_lo)
    wg4[3] = wg4[2]
    # wpack: w1t | w2t | wept along columns
    wpack = np.empty((_K, _K + _H + _D), np.float32)
    wpack[:, 0:_K] = (l1_w.astype(np.float64).T * C64[:, None]).astype(np.float32)
    wpack[:, _K:_K + _H] = l2_w.T
    wpack[:, _K + _H:_K + _H + _D] = (
        ep_w.astype(np.float64).T * C64[:, None] / 100.0).astype(np.float32)
    bexp = (-means.astype(np.float64) * a64).astype(np.float32)[:, None]

    in_maps = []
    for b in range(_B):
        xv_full = _dist_xv(pos[b], mw, bw, node_type_edge[b])  # [N,N]
        xv = np.full(_NPAIR, _DUMMY_XV, np.float32)
        pl = pad_lists[b]
        if len(pl):
            # row slab: pair p = s*N + j  ->  (i=pl[s], j)
            xv[: len(pl) * _N] = xv_full[pl, :].reshape(-1)
            # col slab: pair p = NPR + i*P + t  ->  (i, j=pl[t])
            col = np.full((_N, _P), _DUMMY_XV, np.float32)
            col[:, : len(pl)] = xv_full[:, pl]
            xv[_NPR:] = col.reshape(-1)
        xv_hi = _round_mantissa(xv)
        xv_lo = _round_mantissa(xv - xv_hi)
        wrhs = np.empty((4, _K + _NPAIR), np.float32)
        wrhs[:, 0:_K] = wg4
        wrhs[0, _K:] = xv_hi
        wrhs[1, _K:] = xv_lo
        wrhs[2, _K:] = xv_hi
        wrhs[3, _K:] = xv_lo
        in_maps.append({"wrhs": wrhs, "wpack": wpack, "bexp": bexp})

    from concourse import bass_utils
    nc = _get_nc()
    res = bass_utils.run_bass_kernel_spmd(nc, in_maps,
                                          core_ids=list(range(_B)))

    dist_emb = np.zeros((_B, _N, _D), np.float32)
    attn = np.zeros((_B, _H, _N, _N), np.float32)
    for b in range(_B):
        r = res.results[b]
        acp = r["attn_d"]                                  # [128, NPAIR//3]
        ac = np.empty((_H, _NPAIR), np.float32)
        for c in range(_NPAIR // _CHUNK):
            ac[:, c * _CHUNK:(c + 1) * _CHUNK] = acp[
                _H * (c % 4):_H * (c % 4 + 1),
                (c // 4) * _CHUNK:(c // 4 + 1) * _CHUNK]
        dist_emb[b] = r["de_d"]
        pl = pad_lists[b]
        for s, idx in enumerate(pl):
            dist_emb[b, idx] = r["dep_d"][s]
            attn[b, :, idx, :] = ac[:, s * _N:(s + 1) * _N]
        for t, idx in enumerate(pl):
            attn[b, :, :, idx] = ac[:, _NPR + t::_P]
    return dist_emb, attn
